# revision 3
# baseline (speedup 1.0000x reference)
"""Trainium2 Bass kernel for nn_DWTExtractor: 2-level Haar DWT + bilinear 2x upsample.

Input  x: (32, 1, 1024, 1024) fp32
Output y: (32, 6, 512, 512) fp32 = [cH1, cV1, cD1, cH2u, cV2u, cD2u]

Sharding: pure batch data-parallel, 4 images per core across 8 cores.

Design: fp16 end-to-end on device; host converts f32->fp16 (with the
1/2 Haar normalization folded in) and back. This halves HBM traffic, which
is the roofline for this kernel.

Layout trick: the input image is loaded as [(p s) w -> p s w] with s=8, so
partition p holds 8 consecutive image rows. All vertical (row-direction)
Haar pairing for BOTH levels then becomes elementwise ops over free-dim
slices (no PE, no transpose):
  level-1 row pair (2k, 2k+1) lives at slices (s=2k, s=2k+1) of partition p
  -> L1 rows r = 4p + k; level-2 pairs rows (4p+2j, 4p+2j+1), same partition.
Horizontal pairing is strided free-dim elementwise. Only the bilinear
H-upsample (which mixes rows across partitions) uses the PE, as 2
accumulating near-diagonal matmuls per output row-slot (weights carry the
1/8 de-scale: W-upsample values are produced 8x true).

Output rows r = 4p + u map to dram as [(p u) w]: each partition writes 4
consecutive rows = contiguous lines.
"""

import numpy as np

import concourse.bass as bass
import concourse.tile as tile
import concourse.mybir as mybir
from concourse import bacc, bass_utils

F32 = mybir.dt.float32
F16 = mybir.dt.float16
AL = mybir.AluOpType

B, H, W = 32, 1024, 1024
NCORES = 8
IMG = B // NCORES  # images per core
HL, WL = H // 2, W // 2  # 512
P = 128


def _build_wm() -> np.ndarray:
    """(128, 8*128) fp16 lhsT blocks W[u][j]: H-upsample taps, x1/8 folded.

    out[q] (upsampled row m = 4q+u) = 0.75*src[m//2] + 0.25*src[m//2 -+ 1],
    src row k lives at (partition k//2, j-slice k%2) of the WU tile.
    lhsT[src_partition, out_partition].
    """
    w = np.zeros((P, 8 * P), np.float32)
    t, qtr = 0.75 / 8, 0.25 / 8
    for u in range(4):
        blk = [np.zeros((P, P), np.float32), np.zeros((P, P), np.float32)]
        for q in range(P):
            m = 4 * q + u
            k0 = m // 2
            blk[k0 % 2][k0 // 2, q] += t
            k1 = k0 - 1 if u % 2 == 0 else k0 + 1
            k1 = min(max(k1, 0), 2 * P - 1)
            blk[k1 % 2][k1 // 2, q] += qtr
        w[:, (2 * u) * P : (2 * u + 1) * P] = blk[0]
        w[:, (2 * u + 1) * P : (2 * u + 2) * P] = blk[1]
    return w.astype(np.float16)


def build_nc() -> "bacc.Bacc":
    nc = bacc.Bacc(
        "TRN2", target_bir_lowering=False, debug=False, num_devices=NCORES,
        name="dwt_extractor2",
    )
    x_d = nc.dram_tensor("xc", [IMG, H, W], F16, kind="ExternalInput")
    wm_d = nc.dram_tensor("wm", [P, 8 * P], F16, kind="ExternalInput")
    y_d = nc.dram_tensor("yc", [IMG, 6, HL, WL], F16, kind="ExternalOutput")

    with tile.TileContext(nc) as tc:
        with (
            tc.tile_pool(name="consts", bufs=1) as cpool,
            tc.tile_pool(name="xin", bufs=2) as xpool,
            tc.tile_pool(name="xq0", bufs=1) as xq0pool,
            tc.tile_pool(name="sv", bufs=2) as svpool,
            tc.tile_pool(name="dv", bufs=2) as dvpool,
            tc.tile_pool(name="ca", bufs=2) as capool,
            tc.tile_pool(name="l2", bufs=2) as l2pool,
            tc.tile_pool(name="b3", bufs=2) as b3pool,
            tc.tile_pool(name="wu", bufs=2) as wupool,
            tc.tile_pool(name="stg1", bufs=2) as stg1pool,
            tc.tile_pool(name="stg2", bufs=2) as stg2pool,
            tc.tile_pool(name="ps", bufs=4, space="PSUM") as pspool,
        ):
            wm = cpool.tile([P, 8 * P], F16)
            nc.scalar.dma_start(wm[:], wm_d[:])
            WT = [wm[:, i * P : (i + 1) * P] for i in range(8)]

            def do_image(b):
                # ---- input in 2 halves: partition p <- image rows 8p..8p+7
                xr_d = x_d[b].rearrange("(p s) w -> p s w", s=8)
                nq = 4 if b == 0 else 2
                sq = 8 // nq
                Xh = []
                for h in range(nq):
                    pool = xq0pool if b == 0 else xpool
                    xt = pool.tile([P, sq * W], F16, tag=f"x{h}.{nq}")
                    xtr = xt[:].rearrange("p (s w) -> p s w", s=sq)
                    if b == 0 and h == 0:
                        nc.sync.dma_start(xtr[:, :, 0 : W // 2], xr_d[:, 0:sq, 0 : W // 2])
                        nc.sync.dma_start(xtr[:, :, W // 2 :], xr_d[:, 0:sq, W // 2 :])
                    else:
                        nc.sync.dma_start(xtr, xr_d[:, sq * h : sq * h + sq, :])
                    Xh.append(xtr)

                # ---- L1 vertical: S/D[p, k, w] = rows (8p+2k) +- (8p+2k+1)
                SV = svpool.tile([P, 4 * W], F16, tag="sv")
                DV = dvpool.tile([P, 4 * W], F16, tag="dv")
                SVr = SV[:].rearrange("p (k w) -> p k w", k=4)
                DVr = DV[:].rearrange("p (k w) -> p k w", k=4)
                kq = sq // 2
                for h in range(nq):
                    nc.vector.tensor_tensor(
                        SVr[:, kq * h : kq * h + kq], Xh[h][:, 0:sq:2, :],
                        Xh[h][:, 1:sq:2, :], AL.add,
                    )
                for h in range(nq):
                    nc.vector.tensor_tensor(
                        DVr[:, kq * h : kq * h + kq], Xh[h][:, 0:sq:2, :],
                        Xh[h][:, 1:sq:2, :], AL.subtract,
                    )

                # ---- L1 horizontal: bands + cA1 (L1 row r = 4p + k) ----
                STG1 = stg1pool.tile([P, 3 * 4 * WL], F16, tag="s1")
                S1r = STG1[:].rearrange("p (band k w) -> p band k w", band=3, k=4)
                CA = capool.tile([P, 4 * WL], F16, tag="ca")
                CAr = CA[:].rearrange("p (k w) -> p k w", k=4)
                Se, So = SVr[:, :, 0 : W : 2], SVr[:, :, 1 : W : 2]
                De, Do = DVr[:, :, 0 : W : 2], DVr[:, :, 1 : W : 2]
                nc.vector.tensor_tensor(CAr, Se, So, AL.add)
                y1_r = y_d[b, 0:3].rearrange("band (p u) w -> p band u w", u=4)
                nc.gpsimd.tensor_tensor(S1r[:, 1], De, Do, AL.add)       # cV1
                nc.gpsimd.tensor_tensor(S1r[:, 2], De, Do, AL.subtract)  # cD1
                if b < IMG - 1:
                    nc.gpsimd.tensor_tensor(S1r[:, 0], Se, So, AL.subtract)  # cH1
                    nc.scalar.dma_start(y1_r, S1r)
                else:
                    # tail image: stream bands out early, split cH1 DVE/Pool
                    nc.scalar.dma_start(y1_r[:, 1:3], S1r[:, 1:3])
                    nc.vector.tensor_tensor(
                        S1r[:, 0, 0:2], Se[:, 0:2], So[:, 0:2], AL.subtract
                    )
                    nc.gpsimd.tensor_tensor(
                        S1r[:, 0, 2:4], Se[:, 2:4], So[:, 2:4], AL.subtract
                    )
                    nc.scalar.dma_start(y1_r[:, 0], S1r[:, 0])
                del De, Do

                # ---- L2 vertical (L2 row r2 = 2p + j) ----
                L2 = l2pool.tile([P, 2 * 2 * WL], F16, tag="l2")
                S2r = L2[:, 0 : 2 * WL].rearrange("p (j w) -> p j w", j=2)
                D2r = L2[:, 2 * WL :].rearrange("p (j w) -> p j w", j=2)
                nc.vector.tensor_tensor(S2r, CAr[:, 0:4:2, :], CAr[:, 1:4:2, :], AL.add)
                nc.vector.tensor_tensor(D2r, CAr[:, 0:4:2, :], CAr[:, 1:4:2, :], AL.subtract)
                S2e, S2o = S2r[:, :, 0 : WL : 2], S2r[:, :, 1 : WL : 2]
                D2e, D2o = D2r[:, :, 0 : WL : 2], D2r[:, :, 1 : WL : 2]

                # ---- per-band: L2 horizontal -> W-upsample -> H-upsample
                for band in range(3):
                    B3 = b3pool.tile([P, 2 * 256], F16, tag=f"b3{band}")
                    B3r = B3[:].rearrange("p (j w) -> p j w", j=2)
                    if band == 0:
                        nc.vector.tensor_tensor(B3r, S2e, S2o, AL.subtract)  # cH2
                    elif band == 1:
                        nc.vector.tensor_tensor(B3r, D2e, D2o, AL.add)       # cV2
                    else:
                        nc.vector.tensor_tensor(B3r, D2e, D2o, AL.subtract)  # cD2

                    WU = wupool.tile([P, 2 * WL], F16, tag=f"wu{band}")
                    WUr = WU[:].rearrange("p (j w) -> p j w", j=2)
                    nc.vector.scalar_tensor_tensor(
                        WUr[:, :, 2 : WL : 2], B3r[:, :, 1:256], 3.0,
                        B3r[:, :, 0:255], AL.mult, AL.add,
                    )
                    nc.vector.scalar_tensor_tensor(
                        WUr[:, :, 1 : WL - 1 : 2], B3r[:, :, 0:255], 3.0,
                        B3r[:, :, 1:256], AL.mult, AL.add,
                    )
                    nc.vector.tensor_scalar_mul(
                        WUr[:, :, 0 : WL : WL - 1], B3r[:, :, 0 : 256 : 255], 4.0
                    )

                    # H-upsample: psum spans 2 banks; one Act evac per 2 u-slots
                    STG2 = stg2pool.tile([P, 4 * WL], F16, tag=f"s2{band}")
                    S2out = STG2[:].rearrange("p (u w) -> p u w", u=4)
                    for uh in range(2):
                        ps = pspool.tile([P, 2 * WL], F32, tag="up")
                        for du in range(2):
                            u = 2 * uh + du
                            psw = ps[:, du * WL : (du + 1) * WL]
                            nc.tensor.matmul(
                                psw, WT[2 * u], WUr[:, 0, :],
                                start=True, stop=False,
                            )
                            nc.tensor.matmul(
                                psw, WT[2 * u + 1], WUr[:, 1, :],
                                start=False, stop=True,
                            )
                        dst = S2out[:, 2 * uh : 2 * uh + 2, :]
                        if b == IMG - 1 and band == 2 and uh == 1:
                            nc.vector.tensor_copy(dst, ps[:])
                        else:
                            nc.scalar.copy(dst, ps[:])

                    nc.scalar.dma_start(
                        y_d[b, 3 + band].rearrange("(p u) w -> p u w", u=4),
                        S2out,
                    )

            for b in range(IMG):
                do_image(b)

    nc.compile()
    return nc


_NC_CACHE = None
LAST_RESULTS = None


def kernel(**inputs) -> np.ndarray:
    global _NC_CACHE, LAST_RESULTS
    trace = bool(inputs.pop("_trace", False))
    x = np.asarray(inputs["x"])
    assert x.shape == (B, 1, H, W), x.shape
    if _NC_CACHE is None:
        _NC_CACHE = build_nc()
    nc = _NC_CACHE
    # fold the Haar 1/2 normalization into the host-side fp16 conversion
    xh = (x[:, 0].astype(np.float32) * 0.5).astype(np.float16)
    wm = _build_wm()
    in_maps = [
        {"xc": np.ascontiguousarray(xh[IMG * c : IMG * (c + 1)]), "wm": wm}
        for c in range(NCORES)
    ]
    res = bass_utils.run_bass_kernel_spmd(
        nc, in_maps, core_ids=list(range(NCORES)), trace=trace
    )
    LAST_RESULTS = res
    out = np.concatenate([res.results[c]["yc"] for c in range(NCORES)], axis=0)
    return out.astype(np.float32)


if __name__ == "__main__":
    rng = np.random.default_rng(0)
    x = rng.standard_normal((B, 1, H, W), dtype=np.float32)
    y = kernel(x=x)
    print("kernel output:", y.shape, y.dtype)


# revision 4
# speedup vs baseline: 1.0240x; 1.0240x over previous
"""Trainium2 Bass kernel for nn_DWTExtractor: 2-level Haar DWT + bilinear 2x upsample.

Input  x: (32, 1, 1024, 1024) fp32
Output y: (32, 6, 512, 512) fp32 = [cH1, cV1, cD1, cH2u, cV2u, cD2u]

Sharding: pure batch data-parallel, 4 images per core across 8 cores.

Design: fp16 end-to-end on device; host converts f32->fp16 (with the
1/2 Haar normalization folded in) and back. This halves HBM traffic, which
is the roofline for this kernel.

Layout trick: the input image is loaded as [(p s) w -> p s w] with s=8, so
partition p holds 8 consecutive image rows. All vertical (row-direction)
Haar pairing for BOTH levels then becomes elementwise ops over free-dim
slices (no PE, no transpose):
  level-1 row pair (2k, 2k+1) lives at slices (s=2k, s=2k+1) of partition p
  -> L1 rows r = 4p + k; level-2 pairs rows (4p+2j, 4p+2j+1), same partition.
Horizontal pairing is strided free-dim elementwise. Only the bilinear
H-upsample (which mixes rows across partitions) uses the PE, as 2
accumulating near-diagonal matmuls per output row-slot (weights carry the
1/8 de-scale: W-upsample values are produced 8x true).

Output rows r = 4p + u map to dram as [(p u) w]: each partition writes 4
consecutive rows = contiguous lines.
"""

import numpy as np

import concourse.bass as bass
import concourse.tile as tile
import concourse.mybir as mybir
from concourse import bacc, bass_utils

F32 = mybir.dt.float32
F16 = mybir.dt.float16
AL = mybir.AluOpType

B, H, W = 32, 1024, 1024
NCORES = 8
IMG = B // NCORES  # images per core
HL, WL = H // 2, W // 2  # 512
P = 128


def _build_wm() -> np.ndarray:
    """(128, 8*128) fp16 lhsT blocks W[u][j]: H-upsample taps, x1/8 folded.

    out[q] (upsampled row m = 4q+u) = 0.75*src[m//2] + 0.25*src[m//2 -+ 1],
    src row k lives at (partition k//2, j-slice k%2) of the WU tile.
    lhsT[src_partition, out_partition].
    """
    w = np.zeros((P, 8 * P), np.float32)
    t, qtr = 0.75 / 8, 0.25 / 8
    for u in range(4):
        blk = [np.zeros((P, P), np.float32), np.zeros((P, P), np.float32)]
        for q in range(P):
            m = 4 * q + u
            k0 = m // 2
            blk[k0 % 2][k0 // 2, q] += t
            k1 = k0 - 1 if u % 2 == 0 else k0 + 1
            k1 = min(max(k1, 0), 2 * P - 1)
            blk[k1 % 2][k1 // 2, q] += qtr
        w[:, (2 * u) * P : (2 * u + 1) * P] = blk[0]
        w[:, (2 * u + 1) * P : (2 * u + 2) * P] = blk[1]
    return w.astype(np.float16)


def build_nc() -> "bacc.Bacc":
    nc = bacc.Bacc(
        "TRN2", target_bir_lowering=False, debug=False, num_devices=NCORES,
        name="dwt_extractor2",
    )
    x_d = nc.dram_tensor("xc", [IMG, H, W], F16, kind="ExternalInput")
    wm_d = nc.dram_tensor("wm", [P, 8 * P], F16, kind="ExternalInput")
    y_d = nc.dram_tensor("yc", [IMG, 6, HL, WL], F16, kind="ExternalOutput")

    with tile.TileContext(nc) as tc:
        with (
            tc.tile_pool(name="consts", bufs=1) as cpool,
            tc.tile_pool(name="xin", bufs=2) as xpool,
            tc.tile_pool(name="xq0", bufs=1) as xq0pool,
            tc.tile_pool(name="sv", bufs=2) as svpool,
            tc.tile_pool(name="dv", bufs=2) as dvpool,
            tc.tile_pool(name="ca", bufs=2) as capool,
            tc.tile_pool(name="l2", bufs=2) as l2pool,
            tc.tile_pool(name="b3", bufs=2) as b3pool,
            tc.tile_pool(name="wu", bufs=2) as wupool,
            tc.tile_pool(name="stg1", bufs=2) as stg1pool,
            tc.tile_pool(name="stg2", bufs=2) as stg2pool,
            tc.tile_pool(name="ps", bufs=4, space="PSUM") as pspool,
        ):
            wm = cpool.tile([P, 8 * P], F16)
            nc.scalar.dma_start(wm[:], wm_d[:])
            WT = [wm[:, i * P : (i + 1) * P] for i in range(8)]

            def do_image(b):
                # ---- input in 2 halves: partition p <- image rows 8p..8p+7
                xr_d = x_d[b].rearrange("(p s) w -> p s w", s=8)
                nq = 4 if b == 0 else 2
                sq = 8 // nq
                Xh = []
                for h in range(nq):
                    pool = xq0pool if b == 0 else xpool
                    xt = pool.tile([P, sq * W], F16, tag=f"x{h}.{nq}")
                    xtr = xt[:].rearrange("p (s w) -> p s w", s=sq)
                    if b == 0 and h == 0:
                        nc.sync.dma_start(xtr[:, :, 0 : W // 2], xr_d[:, 0:sq, 0 : W // 2])
                        nc.sync.dma_start(xtr[:, :, W // 2 :], xr_d[:, 0:sq, W // 2 :])
                    else:
                        nc.sync.dma_start(xtr, xr_d[:, sq * h : sq * h + sq, :])
                    Xh.append(xtr)

                # ---- L1 vertical: S/D[p, k, w] = rows (8p+2k) +- (8p+2k+1)
                SV = svpool.tile([P, 4 * W], F16, tag="sv")
                DV = dvpool.tile([P, 4 * W], F16, tag="dv")
                SVr = SV[:].rearrange("p (k w) -> p k w", k=4)
                DVr = DV[:].rearrange("p (k w) -> p k w", k=4)
                kq = sq // 2
                for h in range(nq):
                    nc.vector.tensor_tensor(
                        SVr[:, kq * h : kq * h + kq], Xh[h][:, 0:sq:2, :],
                        Xh[h][:, 1:sq:2, :], AL.add,
                    )
                DH = []
                for h in range(nq):
                    op = nc.vector.tensor_tensor(
                        DVr[:, kq * h : kq * h + kq], Xh[h][:, 0:sq:2, :],
                        Xh[h][:, 1:sq:2, :], AL.subtract,
                    )
                    DH.append(op)

                # ---- L1 horizontal: bands + cA1 (L1 row r = 4p + k) ----
                STG1 = stg1pool.tile([P, 3 * 4 * WL], F16, tag="s1")
                S1r = STG1[:].rearrange("p (band k w) -> p band k w", band=3, k=4)
                CA = capool.tile([P, 4 * WL], F16, tag="ca")
                CAr = CA[:].rearrange("p (k w) -> p k w", k=4)
                Se, So = SVr[:, :, 0 : W : 2], SVr[:, :, 1 : W : 2]
                De, Do = DVr[:, :, 0 : W : 2], DVr[:, :, 1 : W : 2]
                nc.vector.tensor_tensor(CAr, Se, So, AL.add)
                y1_r = y_d[b, 0:3].rearrange("band (p u) w -> p band u w", u=4)
                for kk in range(2):
                    ksl = slice(2 * kk, 2 * kk + 2)
                    nc.gpsimd.tensor_tensor(
                        S1r[:, 1, ksl], De[:, ksl], Do[:, ksl], AL.add
                    )  # cV1
                    nc.gpsimd.tensor_tensor(
                        S1r[:, 2, ksl], De[:, ksl], Do[:, ksl], AL.subtract
                    )  # cD1
                    # stream each half-row-block of bands 1-2 immediately
                    nc.scalar.dma_start(y1_r[:, 1:3, ksl], S1r[:, 1:3, ksl])
                if b < IMG - 1:
                    nc.gpsimd.tensor_tensor(S1r[:, 0], Se, So, AL.subtract)  # cH1
                else:
                    # tail image: split cH1 DVE/Pool
                    nc.vector.tensor_tensor(
                        S1r[:, 0, 0:2], Se[:, 0:2], So[:, 0:2], AL.subtract
                    )
                    nc.gpsimd.tensor_tensor(
                        S1r[:, 0, 2:4], Se[:, 2:4], So[:, 2:4], AL.subtract
                    )
                nc.scalar.dma_start(y1_r[:, 0], S1r[:, 0])
                del De, Do

                # ---- L2 vertical (L2 row r2 = 2p + j) ----
                L2 = l2pool.tile([P, 2 * 2 * WL], F16, tag="l2")
                S2r = L2[:, 0 : 2 * WL].rearrange("p (j w) -> p j w", j=2)
                D2r = L2[:, 2 * WL :].rearrange("p (j w) -> p j w", j=2)
                nc.vector.tensor_tensor(S2r, CAr[:, 0:4:2, :], CAr[:, 1:4:2, :], AL.add)
                nc.vector.tensor_tensor(D2r, CAr[:, 0:4:2, :], CAr[:, 1:4:2, :], AL.subtract)
                S2e, S2o = S2r[:, :, 0 : WL : 2], S2r[:, :, 1 : WL : 2]
                D2e, D2o = D2r[:, :, 0 : WL : 2], D2r[:, :, 1 : WL : 2]

                # ---- per-band: L2 horizontal -> W-upsample -> H-upsample
                for band in range(3):
                    B3 = b3pool.tile([P, 2 * 256], F16, tag=f"b3{band}")
                    B3r = B3[:].rearrange("p (j w) -> p j w", j=2)
                    if band == 0:
                        nc.vector.tensor_tensor(B3r, S2e, S2o, AL.subtract)  # cH2
                    elif band == 1:
                        nc.vector.tensor_tensor(B3r, D2e, D2o, AL.add)       # cV2
                    else:
                        nc.vector.tensor_tensor(B3r, D2e, D2o, AL.subtract)  # cD2

                    WU = wupool.tile([P, 2 * WL], F16, tag=f"wu{band}")
                    WUr = WU[:].rearrange("p (j w) -> p j w", j=2)
                    nc.vector.scalar_tensor_tensor(
                        WUr[:, :, 2 : WL : 2], B3r[:, :, 1:256], 3.0,
                        B3r[:, :, 0:255], AL.mult, AL.add,
                    )
                    nc.vector.scalar_tensor_tensor(
                        WUr[:, :, 1 : WL - 1 : 2], B3r[:, :, 0:255], 3.0,
                        B3r[:, :, 1:256], AL.mult, AL.add,
                    )
                    nc.vector.tensor_scalar_mul(
                        WUr[:, :, 0 : WL : WL - 1], B3r[:, :, 0 : 256 : 255], 4.0
                    )

                    # H-upsample: psum spans 2 banks; one Act evac per 2 u-slots
                    STG2 = stg2pool.tile([P, 4 * WL], F16, tag=f"s2{band}")
                    S2out = STG2[:].rearrange("p (u w) -> p u w", u=4)
                    for uh in range(2):
                        ps = pspool.tile([P, 2 * WL], F32, tag="up")
                        for du in range(2):
                            u = 2 * uh + du
                            psw = ps[:, du * WL : (du + 1) * WL]
                            nc.tensor.matmul(
                                psw, WT[2 * u], WUr[:, 0, :],
                                start=True, stop=False,
                            )
                            nc.tensor.matmul(
                                psw, WT[2 * u + 1], WUr[:, 1, :],
                                start=False, stop=True,
                            )
                        dst = S2out[:, 2 * uh : 2 * uh + 2, :]
                        if b == IMG - 1 and band == 2 and uh == 1:
                            nc.vector.tensor_copy(dst, ps[:])
                        else:
                            nc.scalar.copy(dst, ps[:])

                    nc.scalar.dma_start(
                        y_d[b, 3 + band].rearrange("(p u) w -> p u w", u=4),
                        S2out,
                    )

            for b in range(IMG):
                do_image(b)

    nc.compile()
    return nc


_NC_CACHE = None
LAST_RESULTS = None


def kernel(**inputs) -> np.ndarray:
    global _NC_CACHE, LAST_RESULTS
    trace = bool(inputs.pop("_trace", False))
    x = np.asarray(inputs["x"])
    assert x.shape == (B, 1, H, W), x.shape
    if _NC_CACHE is None:
        _NC_CACHE = build_nc()
    nc = _NC_CACHE
    # fold the Haar 1/2 normalization into the host-side fp16 conversion
    xh = (x[:, 0].astype(np.float32) * 0.5).astype(np.float16)
    wm = _build_wm()
    in_maps = [
        {"xc": np.ascontiguousarray(xh[IMG * c : IMG * (c + 1)]), "wm": wm}
        for c in range(NCORES)
    ]
    res = bass_utils.run_bass_kernel_spmd(
        nc, in_maps, core_ids=list(range(NCORES)), trace=trace
    )
    LAST_RESULTS = res
    out = np.concatenate([res.results[c]["yc"] for c in range(NCORES)], axis=0)
    return out.astype(np.float32)


if __name__ == "__main__":
    rng = np.random.default_rng(0)
    x = rng.standard_normal((B, 1, H, W), dtype=np.float32)
    y = kernel(x=x)
    print("kernel output:", y.shape, y.dtype)


# revision 5
# speedup vs baseline: 1.0266x; 1.0025x over previous
"""Trainium2 Bass kernel for nn_DWTExtractor: 2-level Haar DWT + bilinear 2x upsample.

Input  x: (32, 1, 1024, 1024) fp32
Output y: (32, 6, 512, 512) fp32 = [cH1, cV1, cD1, cH2u, cV2u, cD2u]

Sharding: pure batch data-parallel, 4 images per core across 8 cores.

Design: fp16 end-to-end on device; host converts f32->fp16 (with the
1/2 Haar normalization folded in) and back. This halves HBM traffic, which
is the roofline for this kernel.

Layout trick: the input image is loaded as [(p s) w -> p s w] with s=8, so
partition p holds 8 consecutive image rows. All vertical (row-direction)
Haar pairing for BOTH levels then becomes elementwise ops over free-dim
slices (no PE, no transpose):
  level-1 row pair (2k, 2k+1) lives at slices (s=2k, s=2k+1) of partition p
  -> L1 rows r = 4p + k; level-2 pairs rows (4p+2j, 4p+2j+1), same partition.
Horizontal pairing is strided free-dim elementwise. Only the bilinear
H-upsample (which mixes rows across partitions) uses the PE, as 2
accumulating near-diagonal matmuls per output row-slot (weights carry the
1/8 de-scale: W-upsample values are produced 8x true).

Output rows r = 4p + u map to dram as [(p u) w]: each partition writes 4
consecutive rows = contiguous lines.
"""

import numpy as np

import concourse.bass as bass
import concourse.tile as tile
import concourse.mybir as mybir
from concourse import bacc, bass_utils

F32 = mybir.dt.float32
F16 = mybir.dt.float16
AL = mybir.AluOpType

B, H, W = 32, 1024, 1024
NCORES = 8
IMG = B // NCORES  # images per core
HL, WL = H // 2, W // 2  # 512
P = 128


def _build_wm() -> np.ndarray:
    """(128, 8*128) fp16 lhsT blocks W[u][j]: H-upsample taps, x1/8 folded.

    out[q] (upsampled row m = 4q+u) = 0.75*src[m//2] + 0.25*src[m//2 -+ 1],
    src row k lives at (partition k//2, j-slice k%2) of the WU tile.
    lhsT[src_partition, out_partition].
    """
    w = np.zeros((P, 8 * P), np.float32)
    t, qtr = 0.75 / 8, 0.25 / 8
    for u in range(4):
        blk = [np.zeros((P, P), np.float32), np.zeros((P, P), np.float32)]
        for q in range(P):
            m = 4 * q + u
            k0 = m // 2
            blk[k0 % 2][k0 // 2, q] += t
            k1 = k0 - 1 if u % 2 == 0 else k0 + 1
            k1 = min(max(k1, 0), 2 * P - 1)
            blk[k1 % 2][k1 // 2, q] += qtr
        w[:, (2 * u) * P : (2 * u + 1) * P] = blk[0]
        w[:, (2 * u + 1) * P : (2 * u + 2) * P] = blk[1]
    return w.astype(np.float16)


def build_nc() -> "bacc.Bacc":
    nc = bacc.Bacc(
        "TRN2", target_bir_lowering=False, debug=False, num_devices=NCORES,
        name="dwt_extractor2",
    )
    x_d = nc.dram_tensor("xc", [IMG, H, W], F16, kind="ExternalInput")
    wm_d = nc.dram_tensor("wm", [P, 8 * P], F16, kind="ExternalInput")
    y_d = nc.dram_tensor("yc", [IMG, 6, HL, WL], F16, kind="ExternalOutput")

    with tile.TileContext(nc) as tc:
        with (
            tc.tile_pool(name="consts", bufs=1) as cpool,
            tc.tile_pool(name="xin", bufs=2) as xpool,
            tc.tile_pool(name="xq0", bufs=1) as xq0pool,
            tc.tile_pool(name="sv", bufs=2) as svpool,
            tc.tile_pool(name="dv", bufs=2) as dvpool,
            tc.tile_pool(name="ca", bufs=2) as capool,
            tc.tile_pool(name="l2", bufs=2) as l2pool,
            tc.tile_pool(name="b3", bufs=2) as b3pool,
            tc.tile_pool(name="wu", bufs=2) as wupool,
            tc.tile_pool(name="stg1", bufs=2) as stg1pool,
            tc.tile_pool(name="stg2", bufs=2) as stg2pool,
            tc.tile_pool(name="ps", bufs=4, space="PSUM") as pspool,
        ):
            wm = cpool.tile([P, 8 * P], F16)
            nc.scalar.dma_start(wm[:], wm_d[:])
            WT = [wm[:, i * P : (i + 1) * P] for i in range(8)]

            def do_image(b):
                # ---- input in 2 halves: partition p <- image rows 8p..8p+7
                xr_d = x_d[b].rearrange("(p s) w -> p s w", s=8)
                nq = 4 if b == 0 else 2
                sq = 8 // nq
                Xh = []
                for h in range(nq):
                    pool = xq0pool if b == 0 else xpool
                    xt = pool.tile([P, sq * W], F16, tag=f"x{h}.{nq}")
                    xtr = xt[:].rearrange("p (s w) -> p s w", s=sq)
                    if b == 0 and h == 0:
                        nc.sync.dma_start(xtr[:, :, 0 : W // 2], xr_d[:, 0:sq, 0 : W // 2])
                        nc.sync.dma_start(xtr[:, :, W // 2 :], xr_d[:, 0:sq, W // 2 :])
                    else:
                        nc.sync.dma_start(xtr, xr_d[:, sq * h : sq * h + sq, :])
                    Xh.append(xtr)

                # ---- L1 vertical: S/D[p, k, w] = rows (8p+2k) +- (8p+2k+1)
                SV = svpool.tile([P, 4 * W], F16, tag="sv")
                DV = dvpool.tile([P, 4 * W], F16, tag="dv")
                SVr = SV[:].rearrange("p (k w) -> p k w", k=4)
                DVr = DV[:].rearrange("p (k w) -> p k w", k=4)
                kq = sq // 2
                for h in range(nq):
                    nc.vector.tensor_tensor(
                        SVr[:, kq * h : kq * h + kq], Xh[h][:, 0:sq:2, :],
                        Xh[h][:, 1:sq:2, :], AL.add,
                    )
                DH = []
                for h in range(nq):
                    op = nc.vector.tensor_tensor(
                        DVr[:, kq * h : kq * h + kq], Xh[h][:, 0:sq:2, :],
                        Xh[h][:, 1:sq:2, :], AL.subtract,
                    )
                    DH.append(op)

                # ---- L1 horizontal: bands + cA1 (L1 row r = 4p + k) ----
                STG1 = stg1pool.tile([P, 3 * 4 * WL], F16, tag="s1")
                S1r = STG1[:].rearrange("p (band k w) -> p band k w", band=3, k=4)
                CA = capool.tile([P, 4 * WL], F16, tag="ca")
                CAr = CA[:].rearrange("p (k w) -> p k w", k=4)
                Se, So = SVr[:, :, 0 : W : 2], SVr[:, :, 1 : W : 2]
                De, Do = DVr[:, :, 0 : W : 2], DVr[:, :, 1 : W : 2]
                nc.vector.tensor_tensor(CAr, Se, So, AL.add)
                y1_r = y_d[b, 0:3].rearrange("band (p u) w -> p band u w", u=4)
                for kk in range(2):
                    ksl = slice(2 * kk, 2 * kk + 2)
                    nc.gpsimd.tensor_tensor(
                        S1r[:, 1, ksl], De[:, ksl], Do[:, ksl], AL.add
                    )  # cV1
                    nc.gpsimd.tensor_tensor(
                        S1r[:, 2, ksl], De[:, ksl], Do[:, ksl], AL.subtract
                    )  # cD1
                    # stream each half-row-block of bands 1-2 immediately
                    nc.scalar.dma_start(y1_r[:, 1:3, ksl], S1r[:, 1:3, ksl])
                if b < IMG - 1:
                    nc.gpsimd.tensor_tensor(S1r[:, 0], Se, So, AL.subtract)  # cH1
                else:
                    # tail image: split cH1 DVE/Pool
                    nc.vector.tensor_tensor(
                        S1r[:, 0, 0:2], Se[:, 0:2], So[:, 0:2], AL.subtract
                    )
                    nc.gpsimd.tensor_tensor(
                        S1r[:, 0, 2:4], Se[:, 2:4], So[:, 2:4], AL.subtract
                    )
                nc.scalar.dma_start(y1_r[:, 0], S1r[:, 0])
                del De, Do

                # ---- L2 vertical (L2 row r2 = 2p + j) ----
                L2 = l2pool.tile([P, 2 * 2 * WL], F16, tag="l2")
                S2r = L2[:, 0 : 2 * WL].rearrange("p (j w) -> p j w", j=2)
                D2r = L2[:, 2 * WL :].rearrange("p (j w) -> p j w", j=2)
                nc.vector.tensor_tensor(S2r, CAr[:, 0:4:2, :], CAr[:, 1:4:2, :], AL.add)
                nc.vector.tensor_tensor(D2r, CAr[:, 0:4:2, :], CAr[:, 1:4:2, :], AL.subtract)
                S2e, S2o = S2r[:, :, 0 : WL : 2], S2r[:, :, 1 : WL : 2]
                D2e, D2o = D2r[:, :, 0 : WL : 2], D2r[:, :, 1 : WL : 2]

                # ---- per-band: L2 horizontal -> W-upsample -> H-upsample
                for band in range(3):
                    B3 = b3pool.tile([P, 2 * 256], F16, tag=f"b3{band}")
                    B3r = B3[:].rearrange("p (j w) -> p j w", j=2)
                    if band == 0:
                        nc.vector.tensor_tensor(B3r, S2e, S2o, AL.subtract)  # cH2
                    elif band == 1:
                        nc.vector.tensor_tensor(B3r, D2e, D2o, AL.add)       # cV2
                    else:
                        nc.vector.tensor_tensor(B3r, D2e, D2o, AL.subtract)  # cD2

                    WU = wupool.tile([P, 2 * WL], F16, tag=f"wu{band}")
                    WUr = WU[:].rearrange("p (j w) -> p j w", j=2)
                    nc.vector.scalar_tensor_tensor(
                        WUr[:, :, 2 : WL : 2], B3r[:, :, 1:256], 3.0,
                        B3r[:, :, 0:255], AL.mult, AL.add,
                    )
                    nc.vector.scalar_tensor_tensor(
                        WUr[:, :, 1 : WL - 1 : 2], B3r[:, :, 0:255], 3.0,
                        B3r[:, :, 1:256], AL.mult, AL.add,
                    )
                    nc.gpsimd.tensor_scalar_mul(
                        WUr[:, :, 0 : WL : WL - 1], B3r[:, :, 0 : 256 : 255], 4.0
                    )

                    # H-upsample: psum spans 2 banks; one Act evac per 2 u-slots
                    STG2 = stg2pool.tile([P, 4 * WL], F16, tag=f"s2{band}")
                    S2out = STG2[:].rearrange("p (u w) -> p u w", u=4)
                    for uh in range(2):
                        ps = pspool.tile([P, 2 * WL], F32, tag="up")
                        for du in range(2):
                            u = 2 * uh + du
                            psw = ps[:, du * WL : (du + 1) * WL]
                            nc.tensor.matmul(
                                psw, WT[2 * u], WUr[:, 0, :],
                                start=True, stop=False,
                            )
                            nc.tensor.matmul(
                                psw, WT[2 * u + 1], WUr[:, 1, :],
                                start=False, stop=True,
                            )
                        dst = S2out[:, 2 * uh : 2 * uh + 2, :]
                        if b == IMG - 1 and band == 2 and uh == 1:
                            nc.vector.tensor_copy(dst, ps[:])
                        else:
                            nc.scalar.copy(dst, ps[:])

                    nc.scalar.dma_start(
                        y_d[b, 3 + band].rearrange("(p u) w -> p u w", u=4),
                        S2out,
                    )

            for b in range(IMG):
                do_image(b)

    nc.compile()
    return nc


_NC_CACHE = None
LAST_RESULTS = None


def kernel(**inputs) -> np.ndarray:
    global _NC_CACHE, LAST_RESULTS
    trace = bool(inputs.pop("_trace", False))
    x = np.asarray(inputs["x"])
    assert x.shape == (B, 1, H, W), x.shape
    if _NC_CACHE is None:
        _NC_CACHE = build_nc()
    nc = _NC_CACHE
    # fold the Haar 1/2 normalization into the host-side fp16 conversion
    xh = (x[:, 0].astype(np.float32) * 0.5).astype(np.float16)
    wm = _build_wm()
    in_maps = [
        {"xc": np.ascontiguousarray(xh[IMG * c : IMG * (c + 1)]), "wm": wm}
        for c in range(NCORES)
    ]
    res = bass_utils.run_bass_kernel_spmd(
        nc, in_maps, core_ids=list(range(NCORES)), trace=trace
    )
    LAST_RESULTS = res
    out = np.concatenate([res.results[c]["yc"] for c in range(NCORES)], axis=0)
    return out.astype(np.float32)


if __name__ == "__main__":
    rng = np.random.default_rng(0)
    x = rng.standard_normal((B, 1, H, W), dtype=np.float32)
    y = kernel(x=x)
    print("kernel output:", y.shape, y.dtype)
